# revision 9
# baseline (speedup 1.0000x reference)
"""CrossAttentionPool Trainium2 kernel.

Math (per batch b):
    q = r @ Wq.T + bq                     [H, DK]
    scores[h, r] = (q[h] @ Wk_h) . K[r] / sqrt(DK)   (bk folds out of softmax)
    attn = softmax(scores, axis=r)
    ctx[h] = sum_r attn[h, r] * K[r]      (since v = K @ Wv.T + bv and
    pooled = Wo @ (concat_h Wv_h @ ctx[h] + bv) + bo    sum_r attn = 1)

Device does the R-heavy work (scores, exp+rowsum, ctx); host does all
O(B*D^2) folds and the final projections.

Sharding: data-parallel over B across 8 cores (4 batches per core).
"""

import os
import sys
from contextlib import ExitStack

import numpy as np

for _p in ("/opt/trn_rl_repo", "/root/.axon_site/_ro/trn_rl_repo"):
    if os.path.isdir(_p) and _p not in sys.path:
        sys.path.insert(0, _p)

import concourse.bass as bass
import concourse.tile as tile
from concourse import bacc, mybir

B, R, D, H, DK = 32, 8192, 512, 8, 64
NCORES = 8
BPC = B // NCORES      # batches per core = 4
RC = 16                # r-chunks per batch (512 rows each)
NSUB = 4               # 128-row subtiles per r-chunk
NDSC = 4               # 128-wide d-chunks

F32 = mybir.dt.float32

LAST_EXEC_NS = None
LAST_RESULTS = None

_module_cache = {}


def build_module():
    nc = bacc.Bacc(
        "TRN2",
        target_bir_lowering=False,
        debug=False,
        enable_asserts=True,
        num_devices=NCORES,
    )
    k_in = nc.dram_tensor("k_in", [BPC, R, D], F32, kind="ExternalInput").ap()
    wt_in = nc.dram_tensor("wt_in", [128, BPC * NDSC * H], F32, kind="ExternalInput").ap()
    id_in = nc.dram_tensor("id_in", [128, 128], F32, kind="ExternalInput").ap()
    attn_out = nc.dram_tensor("attn_un", [BPC, H, R], F32, kind="ExternalOutput").ap()
    z_out = nc.dram_tensor("z_part", [H, BPC * RC], F32, kind="ExternalOutput").ap()
    ctx_out = nc.dram_tensor("ctx_un", [BPC, H, D], F32, kind="ExternalOutput").ap()

    with ExitStack() as ctx:
        tc = ctx.enter_context(tile.TileContext(nc))
        kpool = ctx.enter_context(tc.tile_pool(name="kpool", bufs=17))
        ktpsum = ctx.enter_context(tc.tile_pool(name="ktpsum", bufs=2, space="PSUM"))
        ktsb = ctx.enter_context(tc.tile_pool(name="ktsb", bufs=4))
        spsum = ctx.enter_context(tc.tile_pool(name="spsum", bufs=2, space="PSUM"))
        atpsum = ctx.enter_context(tc.tile_pool(name="atpsum", bufs=2, space="PSUM"))
        atsb = ctx.enter_context(tc.tile_pool(name="atsb", bufs=2))
        cpsum = ctx.enter_context(tc.tile_pool(name="cpsum", bufs=1, space="PSUM"))
        scpool = ctx.enter_context(tc.tile_pool(name="scpool", bufs=1, space="PSUM"))
        misc = ctx.enter_context(tc.tile_pool(name="misc", bufs=1))
        esb_pool = ctx.enter_context(tc.tile_pool(name="esb_pool", bufs=1))

        wt = misc.tile([128, BPC * NDSC * H], F32, name="wt")
        nc.sync.dma_start(wt[:], wt_in)
        ident = misc.tile([128, 128], F32, name="ident")
        nc.sync.dma_start(ident[:], id_in)
        zp = misc.tile([H, BPC * RC], F32, name="zp")

        # Matmult supports a single sync-wait in the ISA.  Every PE matmul
        # below must therefore depend on at most ONE semaphore.  Two rules:
        #  - scratch "observe" matmuls absorb each DMA-completion wait onto
        #    a throwaway PE instruction, so real matmuls never wait on DMA;
        #  - copy engines are chosen so a matmul's remaining producer +
        #    slot-release deps land on the same engine semaphore.
        scratch = scpool.tile([128, 128], F32, name="scratch")
        nc.tensor.matmul(scratch[:], ident[:], ident[:], start=True, stop=True)
        nc.tensor.matmul(scratch[:], wt[:], ident[:], start=True, stop=True)

        # K[b] rows r = (rc*NSUB + n)*128 + p -> partition p, free (n, d)
        k_re = k_in.rearrange("b (rc n p) d -> b rc p n d", rc=RC, n=NSUB, p=128)

        all_ksb = {}
        for b in range(BPC):
            ksb = []
            for rc in range(RC):
                kt = kpool.tile([128, NSUB, D], F32, name=f"ksb_{b}_{rc}", tag="ksb")
                nc.sync.dma_start(kt[:], k_re[b, rc])
                # observe the DMA on PE so later matmuls reading kt don't
                # need a DMA wait of their own
                nc.tensor.matmul(
                    scratch[0:1, 0:1], kt[:, 0, 0:1], ident[:, 0:1],
                    start=True, stop=True,
                )
                ksb.append(kt)
            all_ksb[b] = ksb

            esb = esb_pool.tile([H, R], F32, name=f"esb_{b}", tag="esb")

            # --- scores + exp, one r-chunk (512 rows) at a time ---
            for rc in range(RC):
                sp = spsum.tile([H, 512], F32, name=f"sp_{b}_{rc}", tag="sp")
                for dsc in range(NDSC):
                    ktp = ktpsum.tile([128, 512], F32, name=f"ktp_{b}_{rc}_{dsc}", tag="ktp")
                    for j in range(NSUB):
                        # transpose as a plain matmul (K_sub.T @ I): avoids
                        # PE transpose-mode switches entirely
                        nc.tensor.matmul(
                            ktp[:, j * 128:(j + 1) * 128],
                            ksb[rc][:, j, dsc * 128:(dsc + 1) * 128],
                            ident[:],
                            start=True, stop=True,
                        )
                    kts = ktsb.tile([128, 512], F32, name=f"kts_{b}_{rc}_{dsc}", tag="kts")
                    if dsc == 0 or (b == 0 and rc == 0):
                        # dsc==0 feeds the start matmul, which also waits on
                        # the sp slot release (exp on ACT) -> keep same sem.
                        # The first r-chunk stays entirely on ACT: the first
                        # DVE-side wait cannot transitively cover PE deps.
                        nc.scalar.copy(kts[:], ktp[:])
                    else:
                        nc.vector.tensor_copy(kts[:], ktp[:])
                    nc.tensor.matmul(
                        sp[:],
                        wt[:, (b * NDSC + dsc) * H:(b * NDSC + dsc + 1) * H],
                        kts[:],
                        start=(dsc == 0),
                        stop=(dsc == NDSC - 1),
                    )
                nc.scalar.activation(
                    esb[:, rc * 512:(rc + 1) * 512],
                    sp[:],
                    mybir.ActivationFunctionType.Exp,
                    accum_out=zp[:, b * RC + rc: b * RC + rc + 1],
                )

            nc.sync.dma_start(attn_out[b], esb[:])

            # --- ctx = exp(scores)^T-weighted sum of K rows ---
            cp = cpsum.tile([H, D], F32, name=f"cp_{b}", tag="cp")
            for rc in range(RC):
                atp = atpsum.tile([128, NSUB * H], F32, name=f"atp_{b}_{rc}", tag="atp")
                for j in range(NSUB):
                    nc.tensor.matmul(
                        atp[:, j * H:(j + 1) * H],
                        esb[:, rc * 512 + j * 128: rc * 512 + (j + 1) * 128],
                        ident[0:H, 0:H],
                        start=True, stop=True,
                    )
                # ACT so the atp slot release + esb producer share a sem
                ats = atsb.tile([128, NSUB * H], F32, name=f"ats_{b}_{rc}", tag="ats")
                nc.scalar.copy(ats[:], atp[:])
                for j in range(NSUB):
                    nc.tensor.matmul(
                        cp[:],
                        ats[:, j * H:(j + 1) * H],
                        ksb[rc][:, j, :],
                        start=(rc == 0 and j == 0),
                        stop=(rc == RC - 1 and j == NSUB - 1),
                    )
            csb = misc.tile([H, D], F32, name=f"csb_{b}", tag="csb", bufs=2)
            nc.scalar.copy(csb[:], cp[:])
            nc.sync.dma_start(ctx_out[b], csb[:])

        nc.sync.dma_start(z_out, zp[:])
    nc.compile()
    return nc


def get_module():
    if "nc" not in _module_cache:
        _module_cache["nc"] = build_module()
    return _module_cache["nc"]


def host_inputs(r, K, Wq, bq, Wk):
    """Per-core device input maps (minus K, added by caller per core)."""
    q = (r.astype(np.float32) @ Wq.T.astype(np.float32) + bq).reshape(B, H, DK)
    # wt[b,h,d] = q[b,h] @ Wk[h*DK:(h+1)*DK, :] / sqrt(DK)
    wt_full = np.einsum(
        "bhj,hjd->bhd", q, Wk.reshape(H, DK, D).astype(np.float32)
    ) / np.sqrt(DK).astype(np.float32)
    wt_cores = []
    for c in range(NCORES):
        arr = wt_full[c * BPC:(c + 1) * BPC].reshape(BPC, H, NDSC, 128)
        wt_cores.append(np.ascontiguousarray(arr.transpose(3, 0, 2, 1).reshape(128, BPC * NDSC * H)))
    return q, wt_cores


def _numpy_reference(r, K, mask, Wq, bq, Wk, bk, Wv, bv, Wo, bo):
    q = (r @ Wq.T + bq).reshape(B, H, DK)
    k = (K @ Wk.T + bk).reshape(B, R, H, DK)
    v = (K @ Wv.T + bv).reshape(B, R, H, DK)
    scores = np.einsum("bhd,brhd->bhr", q, k) / np.sqrt(DK)
    scores = np.where(mask[:, None, :], scores, np.float32(-1e9))
    scores = scores - scores.max(axis=-1, keepdims=True)
    e = np.exp(scores)
    attn = e / e.sum(axis=-1, keepdims=True)
    pooled = np.einsum("bhr,brhd->bhd", attn, v).reshape(B, D)
    pooled = pooled @ Wo.T + bo
    return pooled.astype(np.float32), attn.astype(np.float32)


def kernel(r, K, mask, Wq, bq, Wk, bk, Wv, bv, Wo, bo):
    global LAST_EXEC_NS, LAST_RESULTS
    r = np.asarray(r, np.float32)
    K = np.asarray(K, np.float32)
    mask = np.asarray(mask)
    if not mask.all():
        # masked path never occurs with the spec's all-ones fill; keep a
        # correct fallback anyway
        return _numpy_reference(
            r, K, mask.astype(bool),
            *(np.asarray(x, np.float32) for x in (Wq, bq, Wk, bk, Wv, bv, Wo, bo)),
        )

    from concourse import bass_utils

    q, wt_cores = host_inputs(r, K, Wq, bq, Wk)
    ident = np.eye(128, dtype=np.float32)
    in_maps = []
    for c in range(NCORES):
        in_maps.append({
            "k_in": np.ascontiguousarray(K[c * BPC:(c + 1) * BPC]),
            "wt_in": wt_cores[c],
            "id_in": ident,
        })

    nc = get_module()
    trace = bool(int(os.environ.get("KERNEL_TRACE", "0")))
    res = bass_utils.run_bass_kernel_spmd(
        nc, in_maps, core_ids=list(range(NCORES)), trace=trace,
    )
    LAST_EXEC_NS = res.exec_time_ns
    LAST_RESULTS = res

    Wv = np.asarray(Wv, np.float32)
    bv = np.asarray(bv, np.float32)
    Wo = np.asarray(Wo, np.float32)
    bo = np.asarray(bo, np.float32)

    attn_un = np.concatenate([res.results[c]["attn_un"] for c in range(NCORES)], axis=0)  # [B,H,R]
    z_part = np.stack([res.results[c]["z_part"] for c in range(NCORES)], axis=0)  # [NC,H,BPC*RC]
    ctx_un = np.concatenate([res.results[c]["ctx_un"] for c in range(NCORES)], axis=0)  # [B,H,D]

    Z = z_part.reshape(NCORES, H, BPC, RC).sum(axis=3).transpose(0, 2, 1).reshape(B, H)
    attn = attn_un / Z[:, :, None]
    ctx = ctx_un / Z[:, :, None]

    vp = np.einsum("hid,bhd->bhi", Wv.reshape(H, DK, D), ctx) + bv.reshape(H, DK)
    pooled = vp.reshape(B, D) @ Wo.T + bo
    return pooled.astype(np.float32), attn.astype(np.float32)


# revision 34
# speedup vs baseline: 1.3553x; 1.3553x over previous
"""CrossAttentionPool Trainium2 kernel.

Math (per batch b):
    q = r @ Wq.T + bq                     [H, DK]
    scores[h, r] = (q[h] @ Wk_h) . K[r] / sqrt(DK)   (bk folds out of softmax)
    attn = softmax(scores, axis=r)
    ctx[h] = sum_r attn[h, r] * K[r]      (since v = K @ Wv.T + bv and
    pooled = Wo @ (concat_h Wv_h @ ctx[h] + bv) + bo    sum_r attn = 1)

Device does the R-heavy work (scores, exp+rowsum, ctx); host does all
O(B*D^2) folds and the final projections.

Sharding: data-parallel over B across 8 cores (4 batches per core).
"""

import os
import sys
from contextlib import ExitStack

import numpy as np

for _p in ("/opt/trn_rl_repo", "/root/.axon_site/_ro/trn_rl_repo"):
    if os.path.isdir(_p) and _p not in sys.path:
        sys.path.insert(0, _p)

import concourse.bass as bass
import concourse.tile as tile
from concourse import bacc, mybir

B, R, D, H, DK = 32, 8192, 512, 8, 64
NCORES = 8
BPC = B // NCORES      # batches per core = 4
RC = 16                # r-chunks per batch (512 rows each)
NSUB = 4               # 128-row subtiles per r-chunk
NDSC = 4               # 128-wide d-chunks
G = 4                  # PE column-tile groups (32-col strips)
RCQ = RC // G          # r-chunk quads; rc = rcq*G + g

F32 = mybir.dt.float32

LAST_EXEC_NS = None
LAST_RESULTS = None

_module_cache = {}


def build_module():
    nc = bacc.Bacc(
        "TRN2",
        target_bir_lowering=False,
        debug=False,
        enable_asserts=True,
        num_devices=NCORES,
    )
    k_in = nc.dram_tensor("k_in", [BPC, R, D], F32, kind="ExternalInput").ap()
    # w-tilde, zero-padded from H=8 to 32 rows per (b, dsc) so M=32 matmuls
    # initialize full 32-partition groups of the PSUM bank
    wt_in = nc.dram_tensor("wt_in", [128, BPC * NDSC * 32], F32, kind="ExternalInput").ap()
    id_in = nc.dram_tensor("id_in", [128, 128], F32, kind="ExternalInput").ap()
    attn_out = nc.dram_tensor("attn_un", [BPC, H, R], F32, kind="ExternalOutput").ap()
    z_out = nc.dram_tensor("z_part", [128, BPC * RCQ], F32, kind="ExternalOutput").ap()
    ctx_out = nc.dram_tensor("ctx_un", [BPC, G, H, D], F32, kind="ExternalOutput").ap()

    with ExitStack() as ctx:
        tc = ctx.enter_context(tile.TileContext(nc))
        kpool = ctx.enter_context(tc.tile_pool(name="kpool", bufs=16))
        ktpsum = ctx.enter_context(tc.tile_pool(name="ktpsum", bufs=2, space="PSUM"))
        ktsb = ctx.enter_context(tc.tile_pool(name="ktsb", bufs=17))
        spsum = ctx.enter_context(tc.tile_pool(name="spsum", bufs=2, space="PSUM"))
        atpsum = ctx.enter_context(tc.tile_pool(name="atpsum", bufs=2, space="PSUM"))
        cpsum = ctx.enter_context(tc.tile_pool(name="cpsum", bufs=1, space="PSUM"))
        scpool = ctx.enter_context(tc.tile_pool(name="scpool", bufs=1, space="PSUM"))
        misc = ctx.enter_context(tc.tile_pool(name="misc", bufs=1))
        esb_pool = ctx.enter_context(tc.tile_pool(name="esb_pool", bufs=2))
        esbt_pool = ctx.enter_context(tc.tile_pool(name="esbt_pool", bufs=6))

        wt = misc.tile([128, BPC * NDSC * 32], F32, name="wt")
        nc.sync.dma_start(wt[:], wt_in)
        ident = misc.tile([128, 128], F32, name="ident")
        nc.sync.dma_start(ident[:], id_in)
        zp = misc.tile([128, BPC * RCQ], F32, name="zp")
        # persistent attn^T staging: cols (j,g,0:8) live, cols (j,g,8:32)
        # stay zero forever -> M=32 ctx matmuls initialize full bank groups
        ats_pp = [misc.tile([128, NSUB * G * 32], F32, name=f"ats{i}") for i in range(2)]
        nc.gpsimd.memset(ats_pp[0][:], 0.0)
        nc.gpsimd.memset(ats_pp[1][:], 0.0)

        # Matmult supports a single sync-wait in the ISA.  Every PE matmul
        # below must therefore depend on at most ONE semaphore.  Two rules:
        #  - scratch "observe" matmuls absorb each DMA-completion wait onto
        #    a throwaway PE instruction, so real matmuls never wait on DMA;
        #  - copy engines are chosen so a matmul's remaining producer +
        #    slot-release deps land on the same engine semaphore.
        scratch = scpool.tile([128, 128], F32, name="scratch")
        nc.tensor.matmul(scratch[:], ident[:], ident[:], start=True, stop=True)
        nc.tensor.matmul(scratch[:], wt[:, 0:128], ident[:], start=True, stop=True)

        # K[b] rows r = (rc*NSUB + n)*128 + p -> partition p, free (n, d)
        k_re = k_in.rearrange("b (rc n p) d -> b rc p n d", rc=RC, n=NSUB, p=128)
        # attn dram viewed so partition-group g goes to rows h, chunks rcq*G+g
        attn_re = attn_out.rearrange("b h (rcq g f) -> b g h rcq f", rcq=RCQ, g=G, f=512)

        for b in range(BPC):
            ksb = []
            for rc in range(RC):
                kt = kpool.tile([128, NSUB, D], F32, name=f"ksb_{b}_{rc}", tag="ksb")
                nc.sync.dma_start(kt[:], k_re[b, rc])
                # observe the DMA on PE so later matmuls reading kt don't
                # need a DMA wait of their own
                nc.tensor.matmul(
                    scratch[0:1, 0:1], kt[:, 0, 0:1], ident[:, 0:1],
                    start=True, stop=True,
                )
                ksb.append(kt)

            # --- scores + exp, one quad (4 r-chunks, 2048 rows) at a time ---
            esbt = {}
            for rcq in range(RCQ):
                # all transposes of the quad first: full-width matmuls must
                # not interleave with the column-tiled score matmuls below
                kts_q = {}
                for dsc in range(NDSC):
                    for g in range(G):
                        rc = rcq * G + g
                        ktp = ktpsum.tile([128, 512], F32, name=f"ktp_{b}_{rc}_{dsc}", tag="ktp")
                        for j in range(NSUB):
                            nc.tensor.matmul(
                                ktp[:, j * 128:(j + 1) * 128],
                                ksb[rc][:, j, dsc * 128:(dsc + 1) * 128],
                                ident[:],
                                start=True, stop=True,
                            )
                        kts = ktsb.tile([128, 512], F32, name=f"kts_{b}_{rc}_{dsc}", tag="kts")
                        if dsc == 0 or (b == 0 and rcq == 0):
                            # dsc==0 feeds a start matmul, which also waits
                            # on the sp slot release (exp on ACT) -> same sem
                            nc.scalar.copy(kts[:], ktp[:])
                        else:
                            nc.vector.tensor_copy(kts[:], ktp[:])
                        kts_q[(g, dsc)] = kts
                # 16 column-tiled score matmuls; groups overlap in the array
                sp4 = spsum.tile([128, 512], F32, name=f"sp_{b}_{rcq}", tag="sp")
                for dsc in range(NDSC):
                    for g in range(G):
                        nc.tensor.matmul(
                            sp4[32 * g:32 * g + 32, :],
                            wt[:, (b * NDSC + dsc) * 32:(b * NDSC + dsc + 1) * 32],
                            kts_q[(g, dsc)][:],
                            start=(dsc == 0),
                            stop=(dsc == NDSC - 1),
                            tile_position=(0, 32 * g),
                            # sim's group tracker is partition-blind; the 4
                            # col-groups use disjoint partition ranges
                            skip_group_check=True,
                        )
                # one full-bank exp: reads every group's region, so it waits
                # for ALL column-groups' matmuls (PSUM bank R/W hazard)
                esq = esb_pool.tile([128, 512], F32, name=f"esb_{b}_{rcq}", tag="esb")
                nc.scalar.activation(
                    esq[:],
                    sp4[:],
                    mybir.ActivationFunctionType.Exp,
                    accum_out=zp[:, b * RCQ + rcq: b * RCQ + rcq + 1],
                )
                for g in range(G):
                    nc.sync.dma_start(attn_re[b, g][:, rcq, :], esq[32 * g:32 * g + H, :])
                    # stage the group's 8 head-rows down to partition base 0:
                    # PE matmuls (lhsT) cannot source non-zero partition bases
                    et = esbt_pool.tile([H, 512], F32, name=f"esbt_{b}_{rcq}_{g}", tag="esbt")
                    nc.sync.dma_start(et[:], esq[32 * g:32 * g + H, :])
                    esbt[rcq * G + g] = et

            # --- ctx = exp(scores)^T-weighted sum of K rows ---
            cp4 = cpsum.tile([128, 512], F32, name=f"cp_{b}", tag="cp")
            for rcq in range(RCQ):
                atp = atpsum.tile([128, NSUB * G * H], F32, name=f"atp_{b}_{rcq}", tag="atp")
                for j in range(NSUB):
                    for g in range(G):
                        nc.tensor.matmul(
                            atp[:, (j * G + g) * H:(j * G + g + 1) * H],
                            esbt[rcq * G + g][:, j * 128:(j + 1) * 128],
                            ident[0:H, 0:H],
                            start=True, stop=True,
                        )
                # ACT so the atp slot release + esb producer share a sem;
                # strided dest: 8 live cols per 32-col block, rest stay zero
                ats = ats_pp[rcq % 2]
                nc.scalar.copy(
                    ats.rearrange("p (jg c) -> p jg c", c=32)[:, :, 0:H],
                    atp.rearrange("p (jg c) -> p jg c", c=H),
                )
                for j in range(NSUB):
                    for g in range(G):
                        rc = rcq * G + g
                        nc.tensor.matmul(
                            cp4[32 * g:32 * g + 32, :],
                            ats[:, (j * G + g) * 32:(j * G + g + 1) * 32],
                            ksb[rc][:, j, :],
                            start=(rcq == 0 and j == 0),
                            stop=(rcq == RCQ - 1 and j == NSUB - 1),
                            tile_position=(0, 32 * g),
                            skip_group_check=True,
                        )
            csb4 = misc.tile([128, 512], F32, name=f"csb_{b}", tag="csb", bufs=2)
            # full-bank copy: waits for every column-group's last matmul
            nc.scalar.copy(csb4[:], cp4[:])
            for g in range(G):
                nc.sync.dma_start(ctx_out[b, g], csb4[32 * g:32 * g + H, :])

        nc.sync.dma_start(z_out, zp[:])
    nc.compile()
    return nc


def get_module():
    if "nc" not in _module_cache:
        _module_cache["nc"] = build_module()
    return _module_cache["nc"]


def host_inputs(r, K, Wq, bq, Wk):
    """Per-core device input maps (minus K, added by caller per core)."""
    q = (r.astype(np.float32) @ Wq.T.astype(np.float32) + bq).reshape(B, H, DK)
    # wt[b,h,d] = q[b,h] @ Wk[h*DK:(h+1)*DK, :] / sqrt(DK)
    wt_full = np.einsum(
        "bhj,hjd->bhd", q, Wk.reshape(H, DK, D).astype(np.float32)
    ) / np.sqrt(DK).astype(np.float32)
    wt_cores = []
    for c in range(NCORES):
        arr = wt_full[c * BPC:(c + 1) * BPC].reshape(BPC, H, NDSC, 128)
        w8 = arr.transpose(3, 0, 2, 1)  # [128, BPC, NDSC, H]
        w32 = np.zeros((128, BPC, NDSC, 32), dtype=np.float32)
        w32[:, :, :, 0:H] = w8
        wt_cores.append(np.ascontiguousarray(w32.reshape(128, BPC * NDSC * 32)))
    return q, wt_cores


def _numpy_reference(r, K, mask, Wq, bq, Wk, bk, Wv, bv, Wo, bo):
    q = (r @ Wq.T + bq).reshape(B, H, DK)
    k = (K @ Wk.T + bk).reshape(B, R, H, DK)
    v = (K @ Wv.T + bv).reshape(B, R, H, DK)
    scores = np.einsum("bhd,brhd->bhr", q, k) / np.sqrt(DK)
    scores = np.where(mask[:, None, :], scores, np.float32(-1e9))
    scores = scores - scores.max(axis=-1, keepdims=True)
    e = np.exp(scores)
    attn = e / e.sum(axis=-1, keepdims=True)
    pooled = np.einsum("bhr,brhd->bhd", attn, v).reshape(B, D)
    pooled = pooled @ Wo.T + bo
    return pooled.astype(np.float32), attn.astype(np.float32)


def kernel(r, K, mask, Wq, bq, Wk, bk, Wv, bv, Wo, bo):
    global LAST_EXEC_NS, LAST_RESULTS
    r = np.asarray(r, np.float32)
    K = np.asarray(K, np.float32)
    mask = np.asarray(mask)
    if not mask.all():
        # masked path never occurs with the spec's all-ones fill; keep a
        # correct fallback anyway
        return _numpy_reference(
            r, K, mask.astype(bool),
            *(np.asarray(x, np.float32) for x in (Wq, bq, Wk, bk, Wv, bv, Wo, bo)),
        )

    from concourse import bass_utils

    q, wt_cores = host_inputs(r, K, Wq, bq, Wk)
    ident = np.eye(128, dtype=np.float32)
    in_maps = []
    for c in range(NCORES):
        in_maps.append({
            "k_in": np.ascontiguousarray(K[c * BPC:(c + 1) * BPC]),
            "wt_in": wt_cores[c],
            "id_in": ident,
        })

    nc = get_module()
    trace = bool(int(os.environ.get("KERNEL_TRACE", "0")))
    res = bass_utils.run_bass_kernel_spmd(
        nc, in_maps, core_ids=list(range(NCORES)), trace=trace,
    )
    LAST_EXEC_NS = res.exec_time_ns
    LAST_RESULTS = res

    Wv = np.asarray(Wv, np.float32)
    bv = np.asarray(bv, np.float32)
    Wo = np.asarray(Wo, np.float32)
    bo = np.asarray(bo, np.float32)

    attn_un = np.concatenate([res.results[c]["attn_un"] for c in range(NCORES)], axis=0)  # [B,H,R]
    z_part = np.stack([res.results[c]["z_part"] for c in range(NCORES)], axis=0)  # [NC,128,BPC*RCQ]
    ctx_un = np.concatenate([res.results[c]["ctx_un"] for c in range(NCORES)], axis=0)  # [B,G,H,D]

    # z_part rows 32g+h, cols b*RCQ+rcq -> Z[b,h] sums over g and rcq
    zs = z_part.reshape(NCORES, G, 32, BPC, RCQ)[:, :, 0:H]  # [NC,G,H,BPC,RCQ]
    Z = zs.sum(axis=(1, 4)).transpose(0, 2, 1).reshape(B, H)
    attn = attn_un / Z[:, :, None]
    ctx = ctx_un.sum(axis=1) / Z[:, :, None]

    vp = np.einsum("hid,bhd->bhi", Wv.reshape(H, DK, D), ctx) + bv.reshape(H, DK)
    pooled = vp.reshape(B, D) @ Wo.T + bo
    return pooled.astype(np.float32), attn.astype(np.float32)


# revision 40
# speedup vs baseline: 2.7766x; 2.0488x over previous
"""CrossAttentionPool Trainium2 kernel.

Math (per batch b):
    q = r @ Wq.T + bq                     [H, DK]
    scores[h, r] = (q[h] @ Wk_h) . K[r] / sqrt(DK)   (bk folds out of softmax)
    attn = softmax(scores, axis=r)
    ctx[h] = sum_r attn[h, r] * K[r]      (since v = K @ Wv.T + bv and
    pooled = Wo @ (concat_h Wv_h @ ctx[h] + bv) + bo    sum_r attn = 1)

Device does the R-heavy work (scores, exp+rowsum, ctx); host does all
O(B*D^2) folds and the final projections.

Sharding: data-parallel over B across 8 cores (4 batches per core).
"""

import os
import sys
from contextlib import ExitStack

import numpy as np

for _p in ("/opt/trn_rl_repo", "/root/.axon_site/_ro/trn_rl_repo"):
    if os.path.isdir(_p) and _p not in sys.path:
        sys.path.insert(0, _p)

import concourse.bass as bass
import concourse.tile as tile
from concourse import bacc, mybir

B, R, D, H, DK = 32, 8192, 512, 8, 64
NCORES = 8
BPC = B // NCORES      # batches per core = 4
RC = 16                # r-chunks per batch (512 rows each)
NSUB = 4               # 128-row subtiles per r-chunk
NDSC = 4               # 128-wide d-chunks
G = 4                  # PE column-tile groups (32-col strips)
RCQ = RC // G          # r-chunk quads; rc = rcq*G + g

F32 = mybir.dt.float32

LAST_EXEC_NS = None
LAST_RESULTS = None

_module_cache = {}


def build_module():
    nc = bacc.Bacc(
        "TRN2",
        target_bir_lowering=False,
        debug=False,
        enable_asserts=True,
        num_devices=NCORES,
    )
    k_in = nc.dram_tensor("k_in", [BPC, R, D], F32, kind="ExternalInput").ap()
    # w-tilde, zero-padded from H=8 to 32 rows per (b, dsc) so M=32 matmuls
    # initialize full 32-partition groups of the PSUM bank
    wt_in = nc.dram_tensor("wt_in", [128, BPC * NDSC * 32], F32, kind="ExternalInput").ap()
    id_in = nc.dram_tensor("id_in", [128, 128], F32, kind="ExternalInput").ap()
    attn_out = nc.dram_tensor("attn_un", [BPC, H, R], F32, kind="ExternalOutput").ap()
    z_out = nc.dram_tensor("z_part", [128, BPC * RCQ], F32, kind="ExternalOutput").ap()
    ctx_out = nc.dram_tensor("ctx_un", [BPC, G, H, D], F32, kind="ExternalOutput").ap()

    with ExitStack() as ctx:
        tc = ctx.enter_context(tile.TileContext(nc))
        kpool = ctx.enter_context(tc.tile_pool(name="kpool", bufs=16))
        ktpsum = ctx.enter_context(tc.tile_pool(name="ktpsum", bufs=2, space="PSUM"))
        ktsb = ctx.enter_context(tc.tile_pool(name="ktsb", bufs=17))
        spsum = ctx.enter_context(tc.tile_pool(name="spsum", bufs=1, space="PSUM"))
        atpsum = ctx.enter_context(tc.tile_pool(name="atpsum", bufs=1, space="PSUM"))
        cpsum = ctx.enter_context(tc.tile_pool(name="cpsum", bufs=1, space="PSUM"))
        scpool = ctx.enter_context(tc.tile_pool(name="scpool", bufs=1, space="PSUM"))
        misc = ctx.enter_context(tc.tile_pool(name="misc", bufs=1))
        esb_pool = ctx.enter_context(tc.tile_pool(name="esb_pool", bufs=2))
        esbt_pool = ctx.enter_context(tc.tile_pool(name="esbt_pool", bufs=6))

        wt = misc.tile([128, BPC * NDSC * 32], F32, name="wt")
        nc.sync.dma_start(wt[:], wt_in)
        ident = misc.tile([128, 128], F32, name="ident")
        nc.sync.dma_start(ident[:], id_in)
        zp = misc.tile([128, BPC * RCQ], F32, name="zp")
        # persistent attn^T staging: cols (j,g,0:8) live, cols (j,g,8:32)
        # stay zero forever -> M=32 ctx matmuls initialize full bank groups
        ats_pp = [misc.tile([128, NSUB * G * 32], F32, name=f"ats{i}") for i in range(2)]
        nc.gpsimd.memset(ats_pp[0][:], 0.0)
        nc.gpsimd.memset(ats_pp[1][:], 0.0)

        # Matmult supports a single sync-wait in the ISA.  Every PE matmul
        # below must therefore depend on at most ONE semaphore.  Two rules:
        #  - scratch "observe" matmuls absorb each DMA-completion wait onto
        #    a throwaway PE instruction, so real matmuls never wait on DMA;
        #  - copy engines are chosen so a matmul's remaining producer +
        #    slot-release deps land on the same engine semaphore.
        scratch = scpool.tile([128, 128], F32, name="scratch")
        nc.tensor.matmul(scratch[:], ident[:], ident[:], start=True, stop=True)
        nc.tensor.matmul(scratch[:], wt[:, 0:128], ident[:], start=True, stop=True)

        # K[b] rows r = (rc*NSUB + n)*128 + p -> partition p, free (n, d)
        k_re = k_in.rearrange("b (rc n p) d -> b rc p n d", rc=RC, n=NSUB, p=128)
        # attn dram viewed so partition-group g goes to rows h, chunks rcq*G+g
        attn_re = attn_out.rearrange("b h (rcq g f) -> b g h rcq f", rcq=RCQ, g=G, f=512)

        for b in range(BPC):
            ksb = []
            for rc in range(RC):
                kt = kpool.tile([128, NSUB, D], F32, name=f"ksb_{b}_{rc}", tag="ksb")
                nc.sync.dma_start(kt[:], k_re[b, rc])
                # observe the DMA on PE so later matmuls reading kt don't
                # need a DMA wait of their own
                nc.tensor.matmul(
                    scratch[0:1, 0:1], kt[:, 0, 0:1], ident[:, 0:1],
                    start=True, stop=True,
                )
                ksb.append(kt)

            # --- scores + exp, one quad (4 r-chunks, 2048 rows) at a time ---
            esbt = {}
            for rcq in range(RCQ):
                # all transposes of the quad first: full-width matmuls must
                # not interleave with the column-tiled score matmuls below
                kts_q = {}
                for dsc in range(NDSC):
                    for g in range(G):
                        rc = rcq * G + g
                        ktp = ktpsum.tile([128, 512], F32, name=f"ktp_{b}_{rc}_{dsc}", tag="ktp")
                        for j in range(NSUB):
                            # plain matmul K_sub.T @ I (transpose-mode
                            # crashes the exec unit on this runtime)
                            nc.tensor.matmul(
                                ktp[:, j * 128:(j + 1) * 128],
                                ksb[rc][:, j, dsc * 128:(dsc + 1) * 128],
                                ident[:],
                                start=True, stop=True,
                            )
                        kts = ktsb.tile([128, 512], F32, name=f"kts_{b}_{rc}_{dsc}", tag="kts")
                        if dsc == 0 or (b == 0 and rcq == 0):
                            # dsc==0 feeds a start matmul, which also waits
                            # on the sp slot release (exp on ACT) -> same sem
                            nc.scalar.copy(kts[:], ktp[:])
                        else:
                            nc.vector.tensor_copy(kts[:], ktp[:])
                        kts_q[(g, dsc)] = kts
                # 16 column-tiled score matmuls; groups overlap in the array.
                # Two PSUM banks (groups 0,1 -> A at bases 0/32; groups 2,3
                # -> B at bases 64/96) double the PSUM drain parallelism.
                spA = spsum.tile([128, 512], F32, name=f"spA_{b}_{rcq}", tag="spA")
                spB = spsum.tile([128, 512], F32, name=f"spB_{b}_{rcq}", tag="spB")
                sp_of = {0: spA, 1: spA, 2: spB, 3: spB}
                for dsc in range(NDSC):
                    for g in range(G):
                        nc.tensor.matmul(
                            sp_of[g][32 * g:32 * g + 32, :],
                            wt[:, (b * NDSC + dsc) * 32:(b * NDSC + dsc + 1) * 32],
                            kts_q[(g, dsc)][:],
                            start=(dsc == 0),
                            stop=(dsc == NDSC - 1),
                            tile_position=(0, 32 * g),
                            # sim's group tracker is partition-blind; the
                            # col-groups use disjoint partition ranges
                            skip_group_check=True,
                        )
                # half-bank exps: each reads its bank's two group regions,
                # so it waits for those groups' matmuls (PSUM R/W hazard)
                esq = esb_pool.tile([128, 512], F32, name=f"esb_{b}_{rcq}", tag="esb")
                nc.scalar.activation(
                    esq[0:64, :],
                    spA[0:64, :],
                    mybir.ActivationFunctionType.Exp,
                    accum_out=zp[0:64, b * RCQ + rcq: b * RCQ + rcq + 1],
                )
                nc.scalar.activation(
                    esq[64:128, :],
                    spB[64:128, :],
                    mybir.ActivationFunctionType.Exp,
                    accum_out=zp[64:128, b * RCQ + rcq: b * RCQ + rcq + 1],
                )
                for g in range(G):
                    nc.sync.dma_start(attn_re[b, g][:, rcq, :], esq[32 * g:32 * g + H, :])
                    # stage the group's 8 head-rows down to partition base 0:
                    # PE matmuls (lhsT) cannot source non-zero partition bases
                    et = esbt_pool.tile([H, 512], F32, name=f"esbt_{b}_{rcq}_{g}", tag="esbt")
                    nc.sync.dma_start(et[:], esq[32 * g:32 * g + H, :])
                    esbt[rcq * G + g] = et

            # --- ctx = exp(scores)^T-weighted sum of K rows ---
            cpA = cpsum.tile([128, 512], F32, name=f"cpA_{b}", tag="cpA")
            cpB = cpsum.tile([128, 512], F32, name=f"cpB_{b}", tag="cpB")
            cp_of = {0: cpA, 1: cpA, 2: cpB, 3: cpB}
            for rcq in range(RCQ):
                atp = atpsum.tile([128, NSUB * G * H], F32, name=f"atp_{b}_{rcq}", tag="atp")
                for j in range(NSUB):
                    for g in range(G):
                        nc.tensor.matmul(
                            atp[:, (j * G + g) * H:(j * G + g + 1) * H],
                            esbt[rcq * G + g][:, j * 128:(j + 1) * 128],
                            ident[0:H, 0:H],
                            start=True, stop=True,
                        )
                # ACT so the atp slot release + esb producer share a sem;
                # strided dest: 8 live cols per 32-col block, rest stay zero
                ats = ats_pp[rcq % 2]
                nc.scalar.copy(
                    ats.rearrange("p (jg c) -> p jg c", c=32)[:, :, 0:H],
                    atp.rearrange("p (jg c) -> p jg c", c=H),
                )
                for j in range(NSUB):
                    for g in range(G):
                        rc = rcq * G + g
                        nc.tensor.matmul(
                            cp_of[g][32 * g:32 * g + 32, :],
                            ats[:, (j * G + g) * 32:(j * G + g + 1) * 32],
                            ksb[rc][:, j, :],
                            start=(rcq == 0 and j == 0),
                            stop=(rcq == RCQ - 1 and j == NSUB - 1),
                            tile_position=(0, 32 * g),
                            skip_group_check=True,
                        )
            csb4 = misc.tile([128, 512], F32, name=f"csb_{b}", tag="csb", bufs=2)
            # half-bank copies: each waits for its bank's group matmuls
            nc.scalar.copy(csb4[0:64, :], cpA[0:64, :])
            nc.scalar.copy(csb4[64:128, :], cpB[64:128, :])
            for g in range(G):
                nc.sync.dma_start(ctx_out[b, g], csb4[32 * g:32 * g + H, :])

        nc.sync.dma_start(z_out, zp[:])
    nc.compile()
    return nc


def get_module():
    if "nc" not in _module_cache:
        _module_cache["nc"] = build_module()
    return _module_cache["nc"]


def host_inputs(r, K, Wq, bq, Wk):
    """Per-core device input maps (minus K, added by caller per core)."""
    q = (r.astype(np.float32) @ Wq.T.astype(np.float32) + bq).reshape(B, H, DK)
    # wt[b,h,d] = q[b,h] @ Wk[h*DK:(h+1)*DK, :] / sqrt(DK)
    wt_full = np.einsum(
        "bhj,hjd->bhd", q, Wk.reshape(H, DK, D).astype(np.float32)
    ) / np.sqrt(DK).astype(np.float32)
    wt_cores = []
    for c in range(NCORES):
        arr = wt_full[c * BPC:(c + 1) * BPC].reshape(BPC, H, NDSC, 128)
        w8 = arr.transpose(3, 0, 2, 1)  # [128, BPC, NDSC, H]
        w32 = np.zeros((128, BPC, NDSC, 32), dtype=np.float32)
        w32[:, :, :, 0:H] = w8
        wt_cores.append(np.ascontiguousarray(w32.reshape(128, BPC * NDSC * 32)))
    return q, wt_cores


def _numpy_reference(r, K, mask, Wq, bq, Wk, bk, Wv, bv, Wo, bo):
    q = (r @ Wq.T + bq).reshape(B, H, DK)
    k = (K @ Wk.T + bk).reshape(B, R, H, DK)
    v = (K @ Wv.T + bv).reshape(B, R, H, DK)
    scores = np.einsum("bhd,brhd->bhr", q, k) / np.sqrt(DK)
    scores = np.where(mask[:, None, :], scores, np.float32(-1e9))
    scores = scores - scores.max(axis=-1, keepdims=True)
    e = np.exp(scores)
    attn = e / e.sum(axis=-1, keepdims=True)
    pooled = np.einsum("bhr,brhd->bhd", attn, v).reshape(B, D)
    pooled = pooled @ Wo.T + bo
    return pooled.astype(np.float32), attn.astype(np.float32)


def kernel(r, K, mask, Wq, bq, Wk, bk, Wv, bv, Wo, bo):
    global LAST_EXEC_NS, LAST_RESULTS
    r = np.asarray(r, np.float32)
    K = np.asarray(K, np.float32)
    mask = np.asarray(mask)
    if not mask.all():
        # masked path never occurs with the spec's all-ones fill; keep a
        # correct fallback anyway
        return _numpy_reference(
            r, K, mask.astype(bool),
            *(np.asarray(x, np.float32) for x in (Wq, bq, Wk, bk, Wv, bv, Wo, bo)),
        )

    from concourse import bass_utils

    q, wt_cores = host_inputs(r, K, Wq, bq, Wk)
    ident = np.eye(128, dtype=np.float32)
    in_maps = []
    for c in range(NCORES):
        in_maps.append({
            "k_in": np.ascontiguousarray(K[c * BPC:(c + 1) * BPC]),
            "wt_in": wt_cores[c],
            "id_in": ident,
        })

    nc = get_module()
    trace = bool(int(os.environ.get("KERNEL_TRACE", "0")))
    res = bass_utils.run_bass_kernel_spmd(
        nc, in_maps, core_ids=list(range(NCORES)), trace=trace,
    )
    LAST_EXEC_NS = res.exec_time_ns
    LAST_RESULTS = res

    Wv = np.asarray(Wv, np.float32)
    bv = np.asarray(bv, np.float32)
    Wo = np.asarray(Wo, np.float32)
    bo = np.asarray(bo, np.float32)

    attn_un = np.concatenate([res.results[c]["attn_un"] for c in range(NCORES)], axis=0)  # [B,H,R]
    z_part = np.stack([res.results[c]["z_part"] for c in range(NCORES)], axis=0)  # [NC,128,BPC*RCQ]
    ctx_un = np.concatenate([res.results[c]["ctx_un"] for c in range(NCORES)], axis=0)  # [B,G,H,D]

    # z_part rows 32g+h, cols b*RCQ+rcq -> Z[b,h] sums over g and rcq
    zs = z_part.reshape(NCORES, G, 32, BPC, RCQ)[:, :, 0:H]  # [NC,G,H,BPC,RCQ]
    Z = zs.sum(axis=(1, 4)).transpose(0, 2, 1).reshape(B, H)
    attn = attn_un / Z[:, :, None]
    ctx = ctx_un.sum(axis=1) / Z[:, :, None]

    vp = np.einsum("hid,bhd->bhi", Wv.reshape(H, DK, D), ctx) + bv.reshape(H, DK)
    pooled = vp.reshape(B, D) @ Wo.T + bo
    return pooled.astype(np.float32), attn.astype(np.float32)


# revision 46
# speedup vs baseline: 3.0601x; 1.1021x over previous
"""CrossAttentionPool Trainium2 kernel.

Math (per batch b):
    q = r @ Wq.T + bq                     [H, DK]
    scores[h, r] = (q[h] @ Wk_h) . K[r] / sqrt(DK)   (bk folds out of softmax)
    attn = softmax(scores, axis=r)
    ctx[h] = sum_r attn[h, r] * K[r]      (since v = K @ Wv.T + bv and
    pooled = Wo @ (concat_h Wv_h @ ctx[h] + bv) + bo    sum_r attn = 1)

Device does the R-heavy work (scores, exp+rowsum, ctx); host does all
O(B*D^2) folds and the final projections.

Sharding: data-parallel over B across 8 cores (4 batches per core).
"""

import os
import sys
from contextlib import ExitStack

import numpy as np

for _p in ("/opt/trn_rl_repo", "/root/.axon_site/_ro/trn_rl_repo"):
    if os.path.isdir(_p) and _p not in sys.path:
        sys.path.insert(0, _p)

import concourse.bass as bass
import concourse.tile as tile
from concourse import bacc, mybir

B, R, D, H, DK = 32, 8192, 512, 8, 64
NCORES = 8
BPC = B // NCORES      # batches per core = 4
RC = 16                # r-chunks per batch (512 rows each)
NSUB = 4               # 128-row subtiles per r-chunk
NDSC = 4               # 128-wide d-chunks
G = 4                  # PE column-tile groups (32-col strips)
RCQ = RC // G          # r-chunk quads; rc = rcq*G + g

F32 = mybir.dt.float32

LAST_EXEC_NS = None
LAST_RESULTS = None

_module_cache = {}


def build_module():
    nc = bacc.Bacc(
        "TRN2",
        target_bir_lowering=False,
        debug=False,
        enable_asserts=True,
        num_devices=NCORES,
    )
    k_in = nc.dram_tensor("k_in", [BPC, R, D], F32, kind="ExternalInput").ap()
    # w-tilde, zero-padded from H=8 to 32 rows per (b, dsc) so M=32 matmuls
    # initialize full 32-partition groups of the PSUM bank
    wt_in = nc.dram_tensor("wt_in", [128, BPC * NDSC * 32], F32, kind="ExternalInput").ap()
    id_in = nc.dram_tensor("id_in", [128, 128], F32, kind="ExternalInput").ap()
    attn_out = nc.dram_tensor("attn_un", [BPC, H, R], F32, kind="ExternalOutput").ap()
    z_out = nc.dram_tensor("z_part", [128, BPC * RCQ], F32, kind="ExternalOutput").ap()
    ctx_out = nc.dram_tensor("ctx_un", [BPC, G, H, D], F32, kind="ExternalOutput").ap()

    with ExitStack() as ctx:
        tc = ctx.enter_context(tile.TileContext(nc))
        kpool = ctx.enter_context(tc.tile_pool(name="kpool", bufs=16))
        ktpsum = ctx.enter_context(tc.tile_pool(name="ktpsum", bufs=2, space="PSUM"))
        ktsb = ctx.enter_context(tc.tile_pool(name="ktsb", bufs=17))
        spsum = ctx.enter_context(tc.tile_pool(name="spsum", bufs=2, space="PSUM"))
        atpsum = ctx.enter_context(tc.tile_pool(name="atpsum", bufs=2, space="PSUM"))
        cpsum = ctx.enter_context(tc.tile_pool(name="cpsum", bufs=1, space="PSUM"))
        scpool = ctx.enter_context(tc.tile_pool(name="scpool", bufs=1, space="PSUM"))
        misc = ctx.enter_context(tc.tile_pool(name="misc", bufs=1))
        esb_pool = ctx.enter_context(tc.tile_pool(name="esb_pool", bufs=2))
        esbt_pool = ctx.enter_context(tc.tile_pool(name="esbt_pool", bufs=6))

        wt = misc.tile([128, BPC * NDSC * 32], F32, name="wt")
        nc.sync.dma_start(wt[:], wt_in)
        ident = misc.tile([128, 128], F32, name="ident")
        nc.sync.dma_start(ident[:], id_in)
        zp = misc.tile([128, BPC * RCQ], F32, name="zp")
        # persistent attn^T staging: cols (j,g,0:8) live, cols (j,g,8:32)
        # stay zero forever -> M=32 ctx matmuls initialize full bank groups
        ats_pp = [misc.tile([128, NSUB * G * 32], F32, name=f"ats{i}") for i in range(2)]
        nc.gpsimd.memset(ats_pp[0][:], 0.0)
        nc.gpsimd.memset(ats_pp[1][:], 0.0)

        # Matmult supports a single sync-wait in the ISA.  Every PE matmul
        # below must therefore depend on at most ONE semaphore.  Two rules:
        #  - scratch "observe" matmuls absorb each DMA-completion wait onto
        #    a throwaway PE instruction, so real matmuls never wait on DMA;
        #  - copy engines are chosen so a matmul's remaining producer +
        #    slot-release deps land on the same engine semaphore.
        scratch = scpool.tile([128, 128], F32, name="scratch")
        nc.tensor.matmul(scratch[:], ident[:], ident[:], start=True, stop=True)
        nc.tensor.matmul(scratch[:], wt[:, 0:128], ident[:], start=True, stop=True)

        # K[b] rows r = (rc*NSUB + n)*128 + p -> partition p, free (n, d)
        k_re = k_in.rearrange("b (rc n p) d -> b rc p n d", rc=RC, n=NSUB, p=128)
        # attn dram viewed so partition-group g goes to rows h, chunks rcq*G+g
        attn_re = attn_out.rearrange("b h (rcq g f) -> b g h rcq f", rcq=RCQ, g=G, f=512)

        for b in range(BPC):
            ksb = []
            for rc in range(RC):
                kt = kpool.tile([128, NSUB, D], F32, name=f"ksb_{b}_{rc}", tag="ksb")
                nc.sync.dma_start(kt[:], k_re[b, rc])
                # observe the DMA on PE so later matmuls reading kt don't
                # need a DMA wait of their own
                nc.tensor.matmul(
                    scratch[0:1, 0:1], kt[:, 0, 0:1], ident[:, 0:1],
                    start=True, stop=True,
                )
                ksb.append(kt)

            # --- scores + exp, one quad (4 r-chunks, 2048 rows) at a time ---
            esbt = {}
            for rcq in range(RCQ):
                # all transposes of the quad first: full-width matmuls must
                # not interleave with the column-tiled score matmuls below
                kts_q = {}
                for dsc in range(NDSC):
                    for g in range(G):
                        rc = rcq * G + g
                        ktp = ktpsum.tile([128, 512], F32, name=f"ktp_{b}_{rc}_{dsc}", tag="ktp")
                        for j in range(NSUB):
                            # plain matmul K_sub.T @ I (transpose-mode
                            # crashes the exec unit on this runtime)
                            nc.tensor.matmul(
                                ktp[:, j * 128:(j + 1) * 128],
                                ksb[rc][:, j, dsc * 128:(dsc + 1) * 128],
                                ident[:],
                                start=True, stop=True,
                            )
                        kts = ktsb.tile([128, 512], F32, name=f"kts_{b}_{rc}_{dsc}", tag="kts")
                        if dsc == 0 or (b == 0 and rcq == 0):
                            # dsc==0 feeds a start matmul, which also waits
                            # on the sp slot release (exp on ACT) -> same sem
                            nc.scalar.copy(kts[:], ktp[:])
                        else:
                            nc.vector.tensor_copy(kts[:], ktp[:])
                        kts_q[(g, dsc)] = kts
                # 16 column-tiled score matmuls; groups overlap in the array
                sp4 = spsum.tile([128, 512], F32, name=f"sp_{b}_{rcq}", tag="sp")
                for dsc in range(NDSC):
                    for g in range(G):
                        nc.tensor.matmul(
                            sp4[32 * g:32 * g + 32, :],
                            wt[:, (b * NDSC + dsc) * 32:(b * NDSC + dsc + 1) * 32],
                            kts_q[(g, dsc)][:],
                            start=(dsc == 0),
                            stop=(dsc == NDSC - 1),
                            tile_position=(0, 32 * g),
                            # sim's group tracker is partition-blind; the
                            # col-groups use disjoint partition ranges
                            skip_group_check=True,
                        )
                # one full-bank exp: reads every group's region, so it waits
                # for ALL column-groups' matmuls (PSUM bank R/W hazard)
                esq = esb_pool.tile([128, 512], F32, name=f"esb_{b}_{rcq}", tag="esb")
                nc.scalar.activation(
                    esq[:],
                    sp4[:],
                    mybir.ActivationFunctionType.Exp,
                    accum_out=zp[:, b * RCQ + rcq: b * RCQ + rcq + 1],
                )
                # stage all 4 groups' 8 head-rows down to one base-0 tile:
                # PE matmuls (lhsT) cannot source non-zero partition bases
                et = esbt_pool.tile([G * H, 512], F32, name=f"esbt_{b}_{rcq}", tag="esbt")
                for g in range(G):
                    nc.sync.dma_start(attn_re[b, g][:, rcq, :], esq[32 * g:32 * g + H, :])
                    nc.sync.dma_start(et[g * H:(g + 1) * H, :], esq[32 * g:32 * g + H, :])
                esbt[rcq] = et

            # --- ctx = exp(scores)^T-weighted sum of K rows ---
            cp4 = cpsum.tile([128, 512], F32, name=f"cp_{b}", tag="cp")
            for rcq in range(RCQ):
                atp = atpsum.tile([128, NSUB * G * H], F32, name=f"atp_{b}_{rcq}", tag="atp")
                for j in range(NSUB):
                    # one [32,128]->[128,32] transpose covers all 4 groups;
                    # output cols j*32 + g*8 + h match the ats block layout
                    nc.tensor.matmul(
                        atp[:, j * G * H:(j + 1) * G * H],
                        esbt[rcq][:, j * 128:(j + 1) * 128],
                        ident[0:G * H, 0:G * H],
                        start=True, stop=True,
                    )
                # ACT so the atp slot release + esb producer share a sem;
                # strided dest: 8 live cols per 32-col block, rest stay zero
                ats = ats_pp[rcq % 2]
                nc.scalar.copy(
                    ats.rearrange("p (jg c) -> p jg c", c=32)[:, :, 0:H],
                    atp.rearrange("p (jg c) -> p jg c", c=H),
                )
                for j in range(NSUB):
                    for g in range(G):
                        rc = rcq * G + g
                        nc.tensor.matmul(
                            cp4[32 * g:32 * g + 32, :],
                            ats[:, (j * G + g) * 32:(j * G + g + 1) * 32],
                            ksb[rc][:, j, :],
                            start=(rcq == 0 and j == 0),
                            stop=(rcq == RCQ - 1 and j == NSUB - 1),
                            tile_position=(0, 32 * g),
                            skip_group_check=True,
                        )
            csb4 = misc.tile([128, 512], F32, name=f"csb_{b}", tag="csb", bufs=2)
            # full-bank copy: waits for every column-group's last matmul
            nc.scalar.copy(csb4[:], cp4[:])
            for g in range(G):
                nc.sync.dma_start(ctx_out[b, g], csb4[32 * g:32 * g + H, :])

        nc.sync.dma_start(z_out, zp[:])
    nc.compile()
    return nc


def get_module():
    if "nc" not in _module_cache:
        _module_cache["nc"] = build_module()
    return _module_cache["nc"]


def host_inputs(r, K, Wq, bq, Wk):
    """Per-core device input maps (minus K, added by caller per core)."""
    q = (r.astype(np.float32) @ Wq.T.astype(np.float32) + bq).reshape(B, H, DK)
    # wt[b,h,d] = q[b,h] @ Wk[h*DK:(h+1)*DK, :] / sqrt(DK)
    wt_full = np.einsum(
        "bhj,hjd->bhd", q, Wk.reshape(H, DK, D).astype(np.float32)
    ) / np.sqrt(DK).astype(np.float32)
    wt_cores = []
    for c in range(NCORES):
        arr = wt_full[c * BPC:(c + 1) * BPC].reshape(BPC, H, NDSC, 128)
        w8 = arr.transpose(3, 0, 2, 1)  # [128, BPC, NDSC, H]
        w32 = np.zeros((128, BPC, NDSC, 32), dtype=np.float32)
        w32[:, :, :, 0:H] = w8
        wt_cores.append(np.ascontiguousarray(w32.reshape(128, BPC * NDSC * 32)))
    return q, wt_cores


def _numpy_reference(r, K, mask, Wq, bq, Wk, bk, Wv, bv, Wo, bo):
    q = (r @ Wq.T + bq).reshape(B, H, DK)
    k = (K @ Wk.T + bk).reshape(B, R, H, DK)
    v = (K @ Wv.T + bv).reshape(B, R, H, DK)
    scores = np.einsum("bhd,brhd->bhr", q, k) / np.sqrt(DK)
    scores = np.where(mask[:, None, :], scores, np.float32(-1e9))
    scores = scores - scores.max(axis=-1, keepdims=True)
    e = np.exp(scores)
    attn = e / e.sum(axis=-1, keepdims=True)
    pooled = np.einsum("bhr,brhd->bhd", attn, v).reshape(B, D)
    pooled = pooled @ Wo.T + bo
    return pooled.astype(np.float32), attn.astype(np.float32)


def kernel(r, K, mask, Wq, bq, Wk, bk, Wv, bv, Wo, bo):
    global LAST_EXEC_NS, LAST_RESULTS
    r = np.asarray(r, np.float32)
    K = np.asarray(K, np.float32)
    mask = np.asarray(mask)
    if not mask.all():
        # masked path never occurs with the spec's all-ones fill; keep a
        # correct fallback anyway
        return _numpy_reference(
            r, K, mask.astype(bool),
            *(np.asarray(x, np.float32) for x in (Wq, bq, Wk, bk, Wv, bv, Wo, bo)),
        )

    from concourse import bass_utils

    q, wt_cores = host_inputs(r, K, Wq, bq, Wk)
    ident = np.eye(128, dtype=np.float32)
    in_maps = []
    for c in range(NCORES):
        in_maps.append({
            "k_in": np.ascontiguousarray(K[c * BPC:(c + 1) * BPC]),
            "wt_in": wt_cores[c],
            "id_in": ident,
        })

    nc = get_module()
    trace = bool(int(os.environ.get("KERNEL_TRACE", "0")))
    res = bass_utils.run_bass_kernel_spmd(
        nc, in_maps, core_ids=list(range(NCORES)), trace=trace,
    )
    LAST_EXEC_NS = res.exec_time_ns
    LAST_RESULTS = res

    Wv = np.asarray(Wv, np.float32)
    bv = np.asarray(bv, np.float32)
    Wo = np.asarray(Wo, np.float32)
    bo = np.asarray(bo, np.float32)

    attn_un = np.concatenate([res.results[c]["attn_un"] for c in range(NCORES)], axis=0)  # [B,H,R]
    z_part = np.stack([res.results[c]["z_part"] for c in range(NCORES)], axis=0)  # [NC,128,BPC*RCQ]
    ctx_un = np.concatenate([res.results[c]["ctx_un"] for c in range(NCORES)], axis=0)  # [B,G,H,D]

    # z_part rows 32g+h, cols b*RCQ+rcq -> Z[b,h] sums over g and rcq
    zs = z_part.reshape(NCORES, G, 32, BPC, RCQ)[:, :, 0:H]  # [NC,G,H,BPC,RCQ]
    Z = zs.sum(axis=(1, 4)).transpose(0, 2, 1).reshape(B, H)
    attn = attn_un / Z[:, :, None]
    ctx = ctx_un.sum(axis=1) / Z[:, :, None]

    vp = np.einsum("hid,bhd->bhi", Wv.reshape(H, DK, D), ctx) + bv.reshape(H, DK)
    pooled = vp.reshape(B, D) @ Wo.T + bo
    return pooled.astype(np.float32), attn.astype(np.float32)
